# revision 1
# baseline (speedup 1.0000x reference)
"""Multi-head attention (B=2, S=2048, D=1024, H=16) on 8 trn2 NeuronCores.

Sharding: 2-way batch x 4-way head-group tensor parallel. Core c handles
batch c//4 and heads 4*(c%4) .. 4*(c%4)+3 (a 256-wide feature slice of the
q/k/v projections, and the matching row-slice of the out projection). Each
core emits a full-size [2048, 1024] partial of the output; the host sums the
4 partials per batch and adds the output bias.

On-device dataflow (per core):
  phase A: activations arrive feature-major ([D, S], pre-transposed on the
           host as part of shard/layout prep, like the weights); DMA straight
           to SBUF and project to QT/KT [dq, t] (feature-major) and V [t, dv]
           (token-major). V gets a 64-wide block of ones columns appended so
           the attn.V matmul also produces the softmax denominator
           replicated on psum partitions 64..127.
  phase B: per (q-chunk, head): scoresT[k, q] = KT_h.T @ QT_h on PE (f32r),
           exp(0.125 * s) on ScalarE in [128, 1024]-wide ACTIVATEs (scores
           are small, so no max-subtraction is needed), then
           outT'[128, q] = sum_k V''_h.T @ P. Rows 64..127 are the softmax
           denominator; normalize rows 0..63 via reciprocal_approx_fast +
           multiply on DVE.
           After all 4 heads of a q-chunk: out-projection matmuls for those
           4 token tiles (keeps PE fed while ACT runs exp for the next
           chunk).
"""

import ml_dtypes
import numpy as np

import concourse.bacc as bacc
import concourse.bass as bass
import concourse.mybir as mybir
import concourse.tile as tile
from concourse.bass_interp import get_hw_module
from concourse.bass_utils import run_bass_kernel_spmd
from concourse.masks import make_identity

# problem constants (hardcoded; must match the reference)
B = 2
S = 2048
D = 1024
NH = 16
DH = 64
SCALE = DH ** -0.5

# sharding
N_CORES = 8
HG = 4                # heads per core
F = HG * DH           # 256 projected features per core
CH = 512              # token chunk
NCH = S // CH         # 4 chunks
P = 128
FT = D // P           # 8 feature tiles
MT = F // P           # 2 projected-feature tiles
KT = S // P           # 16 key-token tiles

f32 = mybir.dt.float32
f32r = mybir.dt.float32r
bf16 = mybir.dt.bfloat16
EXP = mybir.ActivationFunctionType.Exp


def _emit(ctx, nc, tc, aps):
    xqT, xkT, xvT, wqT, wkT, wvT, woT, bq2, bk2, bv1, out = aps

    consts = ctx.enter_context(tc.tile_pool(name="consts", bufs=1))
    persist = ctx.enter_context(tc.tile_pool(name="persist", bufs=1))
    # weights / biases to SBUF (q/k/v projection weights are loaded
    # per-input inside phase A to save SBUF)
    wo_sb = consts.tile([P, MT, D], f32r)
    nc.scalar.dma_start(out=wo_sb, in_=woT.rearrange("(m p) e -> p m e", p=P))
    bq_sb = consts.tile([P, MT], f32)
    bk_sb = consts.tile([P, MT], f32)
    nc.scalar.dma_start(out=bq_sb, in_=bq2)
    nc.scalar.dma_start(out=bk_sb, in_=bk2)
    bv_sb = consts.tile([P, F], f32)
    nc.scalar.dma_start(out=bv_sb, in_=bv1.unsqueeze(0).to_broadcast((P, F)))

    # persistent activations
    QT_sb = persist.tile([P, MT, NCH, CH], f32r)   # [dq%128, dq//128, qc, q]
    # KT, zero-padded to full-K contraction: variant par holds head parity
    # par's 64 feature rows, zeros in the other 64. A scores matmul then uses
    # a full [128, 128] stationary operand (K=64 descriptors run at half PE
    # rate), with the zeros annihilating the other head's QT rows.
    KTz_sb = persist.tile([P, 2, MT, NCH, CH], f32r)
    # V'' layout: [k%128, k//128, h, dv | 64 ones columns]
    V_sb = persist.tile([P, KT, HG, P], f32r)
    ones_sb = consts.tile([P, 1], f32)
    nc.vector.memset(ones_sb, 1.0)
    nc.vector.tensor_copy(
        V_sb[:, :, :, DH:P], ones_sb.to_broadcast((P, KT, HG, P - DH))
    )
    zeros_sb = consts.tile([P, 1], f32)
    nc.vector.memset(zeros_sb, 0.0)
    nc.vector.tensor_copy(
        KTz_sb[DH:P, 0], zeros_sb[DH:P].to_broadcast((DH, MT, NCH, CH))
    )
    nc.vector.tensor_copy(
        KTz_sb[0:DH, 1], zeros_sb[0:DH].to_broadcast((DH, MT, NCH, CH))
    )

    with tc.tile_pool(name="w_pool", bufs=2) as w_pool, \
         tc.tile_pool(name="xT_pool", bufs=4) as xT_pool, \
         tc.tile_pool(name="ps_proj", bufs=4, space="PSUM") as ps_proj:
        # phase A: load feature-major x chunks, project. K and Q first so
        # attention score/exp work can begin while V still projects.
        for which, (xT_ap, wT_ap) in enumerate(
            ((xkT, wkT), (xqT, wqT), (xvT, wvT))
        ):  # 0=K, 1=Q, 2=V
            w_sb = w_pool.tile([P, FT, F], f32r, tag="w")
            nc.scalar.dma_start(
                out=w_sb, in_=wT_ap.rearrange("(ft p) m -> p ft m", p=P)
            )
            for c in range(NCH):
                xT = xT_pool.tile([P, FT, CH], f32r, tag="xT")
                nc.sync.dma_start(
                    out=xT,
                    in_=xT_ap[:, c * CH:(c + 1) * CH].rearrange(
                        "(ft p) t -> p ft t", p=P
                    ),
                )
                if which != 2:  # Q / K: feature-major [dq, t]
                    is_q = which == 1
                    b_sb = bq_sb if is_q else bk_sb
                    for m in range(MT):
                        ps = ps_proj.tile([P, CH], f32, tag="proj")
                        for ft in range(FT):
                            nc.tensor.matmul(
                                ps,
                                w_sb[:, ft, m * P:(m + 1) * P],
                                xT[:, ft, :],
                                start=(ft == 0),
                                stop=(ft == FT - 1),
                            )
                        if is_q:
                            nc.vector.tensor_scalar_add(
                                QT_sb[:, m, c, :], ps, b_sb[:, m:m + 1]
                            )
                        else:
                            nc.vector.tensor_scalar_add(
                                KTz_sb[0:DH, 0, m, c, :], ps[0:DH, :],
                                b_sb[0:DH, m:m + 1],
                            )
                            nc.vector.tensor_scalar_add(
                                KTz_sb[DH:P, 1, m, c, :], ps[DH:P, :],
                                b_sb[DH:P, m:m + 1],
                            )
                else:  # V: token-major [t, dv]
                    for t4 in range(CH // P):
                        ps = ps_proj.tile([P, F], f32, tag="proj")
                        for ft in range(FT):
                            nc.tensor.matmul(
                                ps,
                                xT[:, ft, t4 * P:(t4 + 1) * P],
                                w_sb[:, ft, :],
                                start=(ft == 0),
                                stop=(ft == FT - 1),
                            )
                        kt = c * (CH // P) + t4
                        nc.vector.tensor_add(
                            V_sb[:, kt, :, 0:DH],
                            ps.rearrange("p (h d) -> p h d", h=HG),
                            bv_sb.rearrange("p (h d) -> p h d", h=HG),
                        )

    with tc.tile_pool(name="ps_s", bufs=2, space="PSUM") as ps_s, \
         tc.tile_pool(name="ps_o", bufs=2, space="PSUM") as ps_o, \
         tc.tile_pool(name="ps_out", bufs=2, space="PSUM") as ps_out, \
         tc.tile_pool(name="pt_pool", bufs=2) as pt_pool, \
         tc.tile_pool(name="ot_pool", bufs=1) as ot_pool, \
         tc.tile_pool(name="o_stage", bufs=3) as o_stage, \
         tc.tile_pool(name="rc_pool", bufs=1) as rc_pool:
        OT_sb = ot_pool.tile([P, MT, NCH, CH], f32r)
        # phase B: attention per (q-chunk, head), then the chunk's out-proj
        for qc in range(NCH):
            for h in range(HG):
                mh, p0 = divmod(h, 2)
                p0 *= DH
                PT = pt_pool.tile([P, KT, CH], f32r, tag="PT")
                par = h % 2
                for kg in range(KT // 2):  # 2 k-tiles share a psum group
                    ps = ps_s.tile([P, 2, CH], f32, tag="s")
                    for j in range(2):
                        kt = kg * 2 + j
                        nc.tensor.matmul(
                            ps[:, j, :],
                            KTz_sb[:, par, mh, kt // 4,
                                   (kt % 4) * P:(kt % 4) * P + P],
                            QT_sb[:, mh, qc, :],
                            start=True,
                            stop=True,
                        )
                    nc.scalar.activation(
                        out=PT[:, kg * 2:kg * 2 + 2, :], in_=ps,
                        func=EXP, scale=SCALE,
                    )
                po = ps_o.tile([P, CH], f32, tag="o")
                for kt in range(KT):
                    nc.tensor.matmul(
                        po,
                        V_sb[:, kt, h, :],
                        PT[:, kt, :],
                        start=(kt == 0),
                        stop=(kt == KT - 1),
                    )
                rs = rc_pool.tile([DH, CH], f32, tag="rs")
                rc = rc_pool.tile([DH, CH], f32, tag="rc")
                nc.vector.tensor_copy(rs, po[DH:P, :])
                nc.vector.reciprocal_approx_fast(rc, rs)
                nc.vector.tensor_mul(
                    OT_sb[p0:p0 + DH, mh, qc, :], po[0:DH, :], rc
                )
            # out projection for this chunk's 4 token tiles
            for t4 in range(NCH):
                ob = o_stage.tile([P, D], f32, tag="ob")
                for n2 in range(D // CH):
                    ps = ps_out.tile([P, CH], f32, tag="po")
                    for m in range(MT):
                        nc.tensor.matmul(
                            ps,
                            OT_sb[:, m, qc, t4 * P:(t4 + 1) * P],
                            wo_sb[:, m, n2 * CH:(n2 + 1) * CH],
                            start=(m == 0),
                            stop=(m == MT - 1),
                        )
                    nc.vector.tensor_copy(ob[:, n2 * CH:(n2 + 1) * CH], ps)
                tt = qc * NCH + t4
                nc.sync.dma_start(out=out[tt * P:(tt + 1) * P, :], in_=ob)


def _build():
    nc = bacc.Bacc("TRN2", target_bir_lowering=False, debug=False)
    xqT = nc.dram_tensor("xqT", [D, S], f32r, kind="ExternalInput").ap()
    xkT = nc.dram_tensor("xkT", [D, S], f32r, kind="ExternalInput").ap()
    xvT = nc.dram_tensor("xvT", [D, S], f32r, kind="ExternalInput").ap()
    wqT = nc.dram_tensor("wqT", [D, F], f32r, kind="ExternalInput").ap()
    wkT = nc.dram_tensor("wkT", [D, F], f32r, kind="ExternalInput").ap()
    wvT = nc.dram_tensor("wvT", [D, F], f32r, kind="ExternalInput").ap()
    woT = nc.dram_tensor("woT", [F, D], f32r, kind="ExternalInput").ap()
    bq2 = nc.dram_tensor("bq2", [P, MT], f32, kind="ExternalInput").ap()
    bk2 = nc.dram_tensor("bk2", [P, MT], f32, kind="ExternalInput").ap()
    bv1 = nc.dram_tensor("bv1", [F], f32, kind="ExternalInput").ap()
    out = nc.dram_tensor("out", [S, D], f32, kind="ExternalOutput").ap()
    from contextlib import ExitStack

    with tile.TileContext(nc) as tc, ExitStack() as ctx:
        _emit(ctx, nc, tc,
              (xqT, xkT, xvT, wqT, wkT, wvT, woT, bq2, bk2, bv1, out))
    nc.compile()
    nc.m = get_hw_module(nc.m)
    return nc


_cached_nc = None


def _get_nc():
    global _cached_nc
    if _cached_nc is None:
        _cached_nc = _build()
    return _cached_nc


def make_in_maps(query, key, value, Wq, bq, Wk, bk, Wv, bv, Wo, bo):
    query, key, value, Wq, bq, Wk, bk, Wv, bv, Wo = (
        np.asarray(a, np.float32)
        for a in (query, key, value, Wq, bq, Wk, bk, Wv, bv, Wo)
    )
    xTs = [
        tuple(np.ascontiguousarray(a[b].T) for a in (query, key, value))
        for b in range(B)
    ]
    in_maps = []
    for c in range(N_CORES):
        b, g = divmod(c, 4)
        fs = slice(g * F, (g + 1) * F)
        qT, kT, vT = xTs[b]
        in_maps.append({
            "xqT": qT,
            "xkT": kT,
            "xvT": vT,
            "wqT": np.ascontiguousarray(Wq[fs].T),
            "wkT": np.ascontiguousarray(Wk[fs].T),
            "wvT": np.ascontiguousarray(Wv[fs].T),
            "woT": np.ascontiguousarray(Wo[:, fs].T),
            "bq2": np.ascontiguousarray(bq[fs].reshape(MT, P).T),
            "bk2": np.ascontiguousarray(bk[fs].reshape(MT, P).T),
            "bv1": np.ascontiguousarray(bv[fs]),
        })
    return in_maps


def combine_outputs(core_outs, bo):
    bo = np.asarray(bo, np.float32)
    out = np.empty((B, S, D), np.float32)
    for b in range(B):
        acc = core_outs[4 * b].astype(np.float32)
        for g in range(1, 4):
            acc = acc + core_outs[4 * b + g]
        out[b] = acc + bo
    return out


def kernel(query, key, value, Wq, bq, Wk, bk, Wv, bv, Wo, bo, **run_kwargs):
    nc = _get_nc()
    in_maps = make_in_maps(query, key, value, Wq, bq, Wk, bk, Wv, bv, Wo, bo)
    res = run_bass_kernel_spmd(
        nc, in_maps, core_ids=list(range(N_CORES)), **run_kwargs
    )
    out = combine_outputs([r["out"] for r in res.results], bo)
    if run_kwargs:
        kernel.last_results = res
    return out



# revision 8
# speedup vs baseline: 1.2466x; 1.2466x over previous
"""Multi-head attention (B=2, S=2048, D=1024, H=16) on 8 trn2 NeuronCores.

Sharding: 2-way batch x 4-way head-group tensor parallel. Core c handles
batch c//4 and heads 4*(c%4) .. 4*(c%4)+3 (a 256-wide feature slice of the
q/k/v projections, and the matching row-slice of the out projection). Each
core emits a full-size [2048, 1024] bf16 partial of the output; the host
sums the 4 partials per batch and adds the output bias.

All matmul operands are bf16 (inputs/weights cast host-side; PSUM stays
f32). Softmax exp runs on ScalarE, optionally split with a DVE+GpSimd
Schraudolph bit-trick path (exp(x) ~ bitcast(i32(x*2^23/ln2 + magic))).

On-device dataflow (per core):
  - Q/K projected feature-major ([dq, t]); V token-major ([t, dv]) with 64
    ones columns appended so attn.V also yields the softmax denominator on
    psum partitions 64..127.
  - scoresT[k, q] per head via row-packed matmul pairs: head 2mh uses SBUF
    partitions 0..63, head 2mh+1 partitions 64..127; the two K=64 matmuls
    target disjoint PE row groups and run concurrently (2x throughput vs
    the zero-padded K=128 form).
  - exp on ScalarE (or Schraudolph on DVE+GpSimd) into PT bf16; attn.V
    accumulates over 16 k-tiles; rows 64..127 are the denominator;
    normalize via reciprocal_approx_fast + multiply on DVE.
  - The emission order software-pipelines everything: exp for q-chunk 0
    starts ~8us in, while the PE interleaves remaining projections,
    attn.V of earlier chunks, and the out-projection between score tiles
    so neither PE nor ScalarE ever starves.
"""

import ml_dtypes
import numpy as np

import concourse.bacc as bacc
import concourse.bass as bass
import concourse.mybir as mybir
import concourse.tile as tile
from concourse.bass_interp import get_hw_module
from concourse.bass_utils import run_bass_kernel_spmd

# problem constants (hardcoded; must match the reference)
B = 2
S = 2048
D = 1024
NH = 16
DH = 64
SCALE = DH ** -0.5

# sharding
N_CORES = 8
HG = 4                # heads per core
F = HG * DH           # 256 projected features per core
CH = 512              # token chunk
NCH = S // CH         # 4 chunks
P = 128
FT = D // P           # 8 feature tiles
MT = F // P           # 2 projected-feature tiles
KT = S // P           # 16 key-token tiles

f32 = mybir.dt.float32
i32 = mybir.dt.int32
bf16 = mybir.dt.bfloat16
EXP = mybir.ActivationFunctionType.Exp

# k-tiles whose exp runs on DVE+GpSimd via the Schraudolph bit trick
# (the rest run on ScalarE). Empty set = all exp on ScalarE.
SCH_KTS = frozenset()
# exp(x) ~ bitcast(i32(x*A + Bm)); A folds in the softmax scale, Bm the
# Schraudolph magic with the half-ulp floor correction.
SCH_A = SCALE * (1 << 23) / np.log(2.0)
SCH_B = float((127 << 23) - 0.043677448 * (1 << 23) + 0.5)


def _emit(ctx, nc, tc, aps):
    xqT, xkT, xvT, wqT, wkT, wvT, woT, bq2, bk2, bv1, out = aps

    consts = ctx.enter_context(tc.tile_pool(name="consts", bufs=1))
    persist = ctx.enter_context(tc.tile_pool(name="persist", bufs=1))

    # weights / biases to SBUF (scalar queue; done before exp work starts)
    wk_sb = consts.tile([P, FT, F], bf16)
    wq_sb = consts.tile([P, FT, F], bf16)
    wv_sb = consts.tile([P, FT, F], bf16)
    nc.scalar.dma_start(out=wk_sb, in_=wkT.rearrange("(ft p) m -> p ft m", p=P))
    nc.scalar.dma_start(out=wq_sb, in_=wqT.rearrange("(ft p) m -> p ft m", p=P))
    nc.scalar.dma_start(out=wv_sb, in_=wvT.rearrange("(ft p) m -> p ft m", p=P))
    wo_sb = consts.tile([P, MT, D], bf16)
    nc.scalar.dma_start(out=wo_sb, in_=woT.rearrange("(m p) e -> p m e", p=P))
    bq_sb = consts.tile([P, MT], f32)
    bk_sb = consts.tile([P, MT], f32)
    nc.scalar.dma_start(out=bq_sb, in_=bq2)
    nc.scalar.dma_start(out=bk_sb, in_=bk2)
    bv_sb = consts.tile([P, F], f32)
    nc.scalar.dma_start(out=bv_sb, in_=bv1.unsqueeze(0).to_broadcast((P, F)))

    # persistent activations
    QT_sb = persist.tile([P, MT, NCH, CH], bf16)   # [dq%128, dq//128, qc, q]
    KT_sb = persist.tile([P, MT, NCH, CH], bf16)   # same layout for K
    # V'' layout: [k%128, k//128, h, dv | 64 ones columns]
    V_sb = persist.tile([P, KT, HG, P], bf16)
    nc.vector.memset(V_sb[:, :, :, DH:P], 1.0)

    # input-chunk DMAs, interleaved so xq c0 lands early (sync queue);
    # xv on the gpsimd queue in parallel
    xk_t, xq_t, xv_t = [], [], []
    xT_pool = ctx.enter_context(tc.tile_pool(name="xT", bufs=2))

    def load_x(eng, xT_ap, c, tag):
        xT = xT_pool.tile([P, FT, CH], bf16, tag=tag)
        eng.dma_start(
            out=xT,
            in_=xT_ap[:, c * CH:(c + 1) * CH].rearrange(
                "(ft p) t -> p ft t", p=P
            ),
        )
        return xT

    xk_t.append(load_x(nc.sync, xkT, 0, "xk"))
    xq_t.append(load_x(nc.sync, xqT, 0, "xq"))
    for c in range(1, NCH):
        xk_t.append(load_x(nc.sync, xkT, c, "xk"))
    for c in range(1, NCH):
        xq_t.append(load_x(nc.sync, xqT, c, "xq"))
    for c in range(NCH):
        xv_t.append(load_x(nc.gpsimd, xvT, c, "xv"))

    ps_proj = ctx.enter_context(
        tc.tile_pool(name="ps_proj", bufs=2, space="PSUM"))
    ps_s = ctx.enter_context(tc.tile_pool(name="ps_s", bufs=2, space="PSUM"))
    ps_o = ctx.enter_context(tc.tile_pool(name="ps_o", bufs=2, space="PSUM"))
    pt_pool = ctx.enter_context(tc.tile_pool(name="pt", bufs=2))
    it_pool = ctx.enter_context(tc.tile_pool(name="it", bufs=2))
    ot_pool = ctx.enter_context(tc.tile_pool(name="ot", bufs=2))
    ob_pool = ctx.enter_context(tc.tile_pool(name="ob", bufs=3))
    rc_pool = ctx.enter_context(tc.tile_pool(name="rc", bufs=1))

    pt_tiles = {}   # (qc, mh) -> PT tile [P, 2, KT, CH]
    ot_tiles = {}   # qc -> OT tile [P, MT, CH]

    def proj_qk(c, is_q):
        xT = (xq_t if is_q else xk_t)[c]
        tgt = QT_sb if is_q else KT_sb
        b_sb = bq_sb if is_q else bk_sb
        w_sb = wq_sb if is_q else wk_sb
        for m in range(MT):
            ps = ps_proj.tile([P, CH], f32, tag="proj")
            for ft in range(FT):
                nc.tensor.matmul(
                    ps, w_sb[:, ft, m * P:(m + 1) * P], xT[:, ft, :],
                    start=(ft == 0), stop=(ft == FT - 1),
                )
            nc.vector.tensor_scalar_add(tgt[:, m, c, :], ps, b_sb[:, m:m + 1])

    def proj_v(c):
        xT = xv_t[c]
        for t4 in range(CH // P):
            ps = ps_proj.tile([P, F], f32, tag="proj")
            for ft in range(FT):
                nc.tensor.matmul(
                    ps, xT[:, ft, t4 * P:(t4 + 1) * P], wv_sb[:, ft, :],
                    start=(ft == 0), stop=(ft == FT - 1),
                )
            kt = c * (CH // P) + t4
            nc.vector.tensor_add(
                V_sb[:, kt, :, 0:DH],
                ps.rearrange("p (h d) -> p h d", h=HG),
                bv_sb.rearrange("p (h d) -> p h d", h=HG),
            )

    def scores(qc, mh, kts):
        pt = pt_tiles[(qc, mh)]
        for kt in kts:
            t0 = (kt % NCH) * P
            ps = ps_s.tile([P, 2, CH], f32, tag="s")
            nc.tensor.matmul(
                ps[:, 0, :], KT_sb[0:DH, mh, kt // NCH, t0:t0 + P],
                QT_sb[0:DH, mh, qc, :], start=True, stop=True,
            )
            nc.tensor.matmul(
                ps[:, 1, :], KT_sb[DH:P, mh, kt // NCH, t0:t0 + P],
                QT_sb[DH:P, mh, qc, :], start=True, stop=True,
            )
            pt_out = pt[:, :, kt, :]
            if kt in SCH_KTS:
                itmp = it_pool.tile([P, 2, CH], i32, tag="it")
                nc.vector.tensor_scalar(
                    itmp, ps, SCH_A, SCH_B,
                    op0=mybir.AluOpType.mult, op1=mybir.AluOpType.add,
                )
                nc.gpsimd.tensor_copy(pt_out, itmp.bitcast(f32))
            else:
                nc.scalar.activation(out=pt_out, in_=ps, func=EXP, scale=SCALE)

    def attn_v(qc, h):
        pt = pt_tiles[(qc, h // 2)]
        po = ps_o.tile([P, CH], f32, tag="o")
        for kt in range(KT):
            nc.tensor.matmul(
                po, V_sb[:, kt, h, :], pt[:, h % 2, kt, :],
                start=(kt == 0), stop=(kt == KT - 1),
            )
        mh, p0 = divmod(h, 2)
        p0 *= DH
        rs = rc_pool.tile([DH, CH], f32, tag="rs")
        rc = rc_pool.tile([DH, CH], f32, tag="rc")
        nc.vector.tensor_copy(rs, po[DH:P, :])
        nc.vector.reciprocal_approx_fast(rc, rs)
        nc.vector.tensor_mul(ot_tiles[qc][p0:p0 + DH, mh, :], po[0:DH, :], rc)

    def out_proj(qc, t4):
        ot = ot_tiles[qc]
        ob = ob_pool.tile([P, D], bf16, tag="ob")
        for n2 in range(D // CH):
            ps = ps_proj.tile([P, CH], f32, tag="proj")
            for m in range(MT):
                nc.tensor.matmul(
                    ps, ot[:, m, t4 * P:(t4 + 1) * P],
                    wo_sb[:, m, n2 * CH:(n2 + 1) * CH],
                    start=(m == 0), stop=(m == MT - 1),
                )
            nc.vector.tensor_copy(ob[:, n2 * CH:(n2 + 1) * CH], ps)
        tt = qc * NCH + t4
        nc.sync.dma_start(out=out[tt * P:(tt + 1) * P, :], in_=ob)

    def new_pt(qc, mh):
        pt_tiles[(qc, mh)] = pt_pool.tile([P, 2, KT, CH], bf16, tag="pt",
                                          name=f"pt{qc}_{mh}")

    def new_ot(qc):
        ot_tiles[qc] = ot_pool.tile([P, MT, CH], bf16, tag="ot",
                                    name=f"ot{qc}")

    # ---- software-pipelined emission order ----
    # Pipeline unit = (qc, mh): 16 score-tile pairs (PE ~3.4us) feeding 16
    # exp ACTIVATEs (ScalarE ~16.5us). Filler work (projections, attn_v of
    # the previous unit, out-proj) is interleaved between score groups so
    # the in-order PE queue never blocks on the exp consumer.
    G = 4  # score k-tiles per emission group

    proj_k = lambda c: (lambda: proj_qk(c, False))
    proj_q = lambda c: (lambda: proj_qk(c, True))
    projv = lambda c: (lambda: proj_v(c))
    sc = lambda qc, mh, g: (
        lambda: scores(qc, mh, range(g * G, (g + 1) * G)))
    av = lambda qc, h: (lambda: attn_v(qc, h))
    op = lambda qc, t4: (lambda: out_proj(qc, t4))
    npt = lambda qc, mh: (lambda: new_pt(qc, mh))
    not_ = lambda qc: (lambda: new_ot(qc))

    schedule = [
        proj_k(0), proj_q(0), npt(0, 0), not_(0),
        # unit (0,0): K/V projections as filler
        sc(0, 0, 0), proj_k(1), sc(0, 0, 1), proj_k(2),
        sc(0, 0, 2), proj_k(3), sc(0, 0, 3), projv(0), npt(0, 1),
        # unit (0,1)
        sc(0, 1, 0), projv(1), sc(0, 1, 1), projv(2),
        sc(0, 1, 2), projv(3), sc(0, 1, 3), proj_q(1),
        npt(1, 0), not_(1),
        # unit (1,0): attn_v(0, h) gated on exp(0, mh) completion
        sc(1, 0, 0), av(0, 0), sc(1, 0, 1), av(0, 1),
        sc(1, 0, 2), proj_q(2), sc(1, 0, 3), npt(1, 1),
        # unit (1,1)
        sc(1, 1, 0), av(0, 2), sc(1, 1, 1), av(0, 3),
        sc(1, 1, 2), op(0, 0), op(0, 1), sc(1, 1, 3), op(0, 2), op(0, 3),
        npt(2, 0), not_(2),
        # unit (2,0)
        sc(2, 0, 0), av(1, 0), sc(2, 0, 1), av(1, 1),
        sc(2, 0, 2), proj_q(3), sc(2, 0, 3), npt(2, 1),
        # unit (2,1)
        sc(2, 1, 0), av(1, 2), sc(2, 1, 1), av(1, 3),
        sc(2, 1, 2), op(1, 0), op(1, 1), sc(2, 1, 3), op(1, 2), op(1, 3),
        npt(3, 0), not_(3),
        # unit (3,0)
        sc(3, 0, 0), av(2, 0), sc(3, 0, 1), av(2, 1),
        sc(3, 0, 2), av(2, 2), sc(3, 0, 3), av(2, 3), npt(3, 1),
        # unit (3,1): attn_v(3, 0/1) overlaps exp(3, mh1)
        sc(3, 1, 0), op(2, 0), sc(3, 1, 1), op(2, 1),
        sc(3, 1, 2), av(3, 0), sc(3, 1, 3), av(3, 1),
        op(2, 2), op(2, 3),
        # tail
        av(3, 2), av(3, 3),
        op(3, 0), op(3, 1), op(3, 2), op(3, 3),
    ]
    for unit in schedule:
        unit()


def _build():
    nc = bacc.Bacc("TRN2", target_bir_lowering=False, debug=False)
    xqT = nc.dram_tensor("xqT", [D, S], bf16, kind="ExternalInput").ap()
    xkT = nc.dram_tensor("xkT", [D, S], bf16, kind="ExternalInput").ap()
    xvT = nc.dram_tensor("xvT", [D, S], bf16, kind="ExternalInput").ap()
    wqT = nc.dram_tensor("wqT", [D, F], bf16, kind="ExternalInput").ap()
    wkT = nc.dram_tensor("wkT", [D, F], bf16, kind="ExternalInput").ap()
    wvT = nc.dram_tensor("wvT", [D, F], bf16, kind="ExternalInput").ap()
    woT = nc.dram_tensor("woT", [F, D], bf16, kind="ExternalInput").ap()
    bq2 = nc.dram_tensor("bq2", [P, MT], f32, kind="ExternalInput").ap()
    bk2 = nc.dram_tensor("bk2", [P, MT], f32, kind="ExternalInput").ap()
    bv1 = nc.dram_tensor("bv1", [F], f32, kind="ExternalInput").ap()
    out = nc.dram_tensor("out", [S, D], bf16, kind="ExternalOutput").ap()
    from contextlib import ExitStack

    with tile.TileContext(nc) as tc, ExitStack() as ctx:
        _emit(ctx, nc, tc,
              (xqT, xkT, xvT, wqT, wkT, wvT, woT, bq2, bk2, bv1, out))
    nc.compile()
    nc.m = get_hw_module(nc.m)
    return nc


_cached_nc = None


def _get_nc():
    global _cached_nc
    if _cached_nc is None:
        _cached_nc = _build()
    return _cached_nc


def make_in_maps(query, key, value, Wq, bq, Wk, bk, Wv, bv, Wo, bo):
    query, key, value, Wq, bq, Wk, bk, Wv, bv, Wo = (
        np.asarray(a, np.float32)
        for a in (query, key, value, Wq, bq, Wk, bk, Wv, bv, Wo)
    )
    bff = ml_dtypes.bfloat16
    xTs = [
        tuple(
            np.ascontiguousarray(a[b].T).astype(bff)
            for a in (query, key, value)
        )
        for b in range(B)
    ]
    in_maps = []
    for c in range(N_CORES):
        b, g = divmod(c, 4)
        fs = slice(g * F, (g + 1) * F)
        qT, kT, vT = xTs[b]
        in_maps.append({
            "xqT": qT,
            "xkT": kT,
            "xvT": vT,
            "wqT": np.ascontiguousarray(Wq[fs].T).astype(bff),
            "wkT": np.ascontiguousarray(Wk[fs].T).astype(bff),
            "wvT": np.ascontiguousarray(Wv[fs].T).astype(bff),
            "woT": np.ascontiguousarray(Wo[:, fs].T).astype(bff),
            "bq2": np.ascontiguousarray(bq[fs].reshape(MT, P).T),
            "bk2": np.ascontiguousarray(bk[fs].reshape(MT, P).T),
            "bv1": np.ascontiguousarray(bv[fs]),
        })
    return in_maps


def combine_outputs(core_outs, bo):
    bo = np.asarray(bo, np.float32)
    out = np.empty((B, S, D), np.float32)
    for b in range(B):
        acc = core_outs[4 * b].astype(np.float32)
        for g in range(1, 4):
            acc = acc + core_outs[4 * b + g].astype(np.float32)
        out[b] = acc + bo
    return out


def kernel(query, key, value, Wq, bq, Wk, bk, Wv, bv, Wo, bo, **run_kwargs):
    nc = _get_nc()
    in_maps = make_in_maps(query, key, value, Wq, bq, Wk, bk, Wv, bv, Wo, bo)
    res = run_bass_kernel_spmd(
        nc, in_maps, core_ids=list(range(N_CORES)), **run_kwargs
    )
    out = combine_outputs([r["out"] for r in res.results], bo)
    if run_kwargs:
        kernel.last_results = res
    return out


# revision 23
# speedup vs baseline: 1.2571x; 1.0084x over previous
"""Multi-head attention (B=2, S=2048, D=1024, H=16) on 8 trn2 NeuronCores.

Sharding: 2-way batch x 4-way head-group tensor parallel. Core c handles
batch c//4 and heads 4*(c%4) .. 4*(c%4)+3 (a 256-wide feature slice of the
q/k/v projections, and the matching row-slice of the out projection). Each
core emits a full-size [2048, 1024] bf16 partial of the output; the host
sums the 4 partials per batch and adds the output bias.

All matmul operands are bf16 (inputs/weights cast host-side; PSUM stays
f32). Softmax exp runs on ScalarE, optionally split with a DVE+GpSimd
Schraudolph bit-trick path (exp(x) ~ bitcast(i32(x*2^23/ln2 + magic))).

On-device dataflow (per core):
  - Q/K projected feature-major ([dq, t]); V token-major ([t, dv]) with 64
    ones columns appended so attn.V also yields the softmax denominator on
    psum partitions 64..127.
  - scoresT[k, q] per head via row-packed matmul pairs: head 2mh uses SBUF
    partitions 0..63, head 2mh+1 partitions 64..127; the two K=64 matmuls
    target disjoint PE row groups and run concurrently (2x throughput vs
    the zero-padded K=128 form).
  - exp on ScalarE (or Schraudolph on DVE+GpSimd) into PT bf16; attn.V
    accumulates over 16 k-tiles; rows 64..127 are the denominator;
    normalize via reciprocal_approx_fast + multiply on DVE.
  - The emission order software-pipelines everything: exp for q-chunk 0
    starts ~8us in, while the PE interleaves remaining projections,
    attn.V of earlier chunks, and the out-projection between score tiles
    so neither PE nor ScalarE ever starves.
"""

import ml_dtypes
import numpy as np

import concourse.bacc as bacc
import concourse.bass as bass
import concourse.mybir as mybir
import concourse.tile as tile
from concourse.bass_interp import get_hw_module
from concourse.bass_utils import run_bass_kernel_spmd

# problem constants (hardcoded; must match the reference)
B = 2
S = 2048
D = 1024
NH = 16
DH = 64
SCALE = DH ** -0.5

# sharding
N_CORES = 8
HG = 4                # heads per core
F = HG * DH           # 256 projected features per core
CH = 512              # token chunk
NCH = S // CH         # 4 chunks
P = 128
FT = D // P           # 8 feature tiles
MT = F // P           # 2 projected-feature tiles
KT = S // P           # 16 key-token tiles

f32 = mybir.dt.float32
i32 = mybir.dt.int32
bf16 = mybir.dt.bfloat16
EXP = mybir.ActivationFunctionType.Exp

# k-tiles whose exp runs on DVE+GpSimd via the Schraudolph bit trick
# (the rest run on ScalarE).
SCH_KTS = frozenset({2, 5, 8, 11, 14})
# exp(x) ~ bitcast(i32(x*A + Bm)); A folds in the softmax scale, Bm the
# Schraudolph magic with the half-ulp floor correction.
SCH_A = SCALE * (1 << 23) / np.log(2.0)
SCH_B = float((127 << 23) - 0.043677448 * (1 << 23) + 0.5)


def _emit(ctx, nc, tc, aps):
    xqT, xkT, xvT, wqT, wkT, wvT, woT, bq2, bk2, bv1, out = aps

    consts = ctx.enter_context(tc.tile_pool(name="consts", bufs=1))
    persist = ctx.enter_context(tc.tile_pool(name="persist", bufs=1))

    # weights / biases to SBUF. DMA issue order = transfer priority order:
    # the critical prefix (wk, biases, wq) on the scalar queue; wv/wo/bv
    # (needed only ~40us in) on the gpsimd queue behind xv.
    wk_sb = consts.tile([P, FT, F], bf16)
    wq_sb = consts.tile([P, FT, F], bf16)
    wv_sb = consts.tile([P, FT, F], bf16)
    wo_sb = consts.tile([P, MT, D], bf16)
    bq_sb = consts.tile([P, MT], f32)
    bk_sb = consts.tile([P, MT], f32)
    bv_sb = consts.tile([P, F], f32)
    nc.scalar.dma_start(out=wk_sb, in_=wkT)
    nc.scalar.dma_start(out=bq_sb, in_=bq2)
    nc.scalar.dma_start(out=bk_sb, in_=bk2)
    nc.scalar.dma_start(out=wq_sb, in_=wqT)

    # persistent activations
    QT_sb = persist.tile([P, MT, NCH, CH], bf16)   # [dq%128, dq//128, qc, q]
    KT_sb = persist.tile([P, MT, NCH, CH], bf16)   # same layout for K
    # V'' layout: [k%128, k//128, h, dv | 64 ones columns]
    V_sb = persist.tile([P, KT, HG, P], bf16)
    nc.vector.memset(V_sb[:, :, :, DH:P], 1.0)

    # input-chunk DMAs, in need order: xk0, xq0 first (sync queue); xv on
    # the gpsimd queue in parallel
    xk_t, xq_t, xv_t = [], [], []
    xT_pool = ctx.enter_context(tc.tile_pool(name="xT", bufs=2))

    def load_x(eng, x4_ap, c, tag, bufs=None):
        xT = xT_pool.tile([P, FT, CH], bf16, tag=tag, bufs=bufs)
        eng.dma_start(out=xT, in_=x4_ap[c])
        return xT

    xk_t.append(load_x(nc.sync, xkT, 0, "xk"))
    xq_t.append(load_x(nc.sync, xqT, 0, "xq", bufs=1))
    for c in range(1, NCH):
        xk_t.append(load_x(nc.sync, xkT, c, "xk"))
    for c in range(1, NCH):
        xq_t.append(load_x(nc.sync, xqT, c, "xq", bufs=1))
    xv_t.append(load_x(nc.gpsimd, xvT, 0, "xv"))
    nc.gpsimd.dma_start(out=bv_sb, in_=bv1.unsqueeze(0).to_broadcast((P, F)))
    nc.gpsimd.dma_start(out=wv_sb, in_=wvT)
    for c in range(1, NCH):
        xv_t.append(load_x(nc.gpsimd, xvT, c, "xv"))
    nc.gpsimd.dma_start(out=wo_sb, in_=woT)

    ps_proj = ctx.enter_context(
        tc.tile_pool(name="ps_proj", bufs=2, space="PSUM"))
    ps_s = ctx.enter_context(tc.tile_pool(name="ps_s", bufs=2, space="PSUM"))
    ps_o = ctx.enter_context(tc.tile_pool(name="ps_o", bufs=2, space="PSUM"))
    pt_pool = ctx.enter_context(tc.tile_pool(name="pt", bufs=3))
    it_pool = ctx.enter_context(tc.tile_pool(name="it", bufs=2))
    ot_pool = ctx.enter_context(tc.tile_pool(name="ot", bufs=2))
    ob_pool = ctx.enter_context(tc.tile_pool(name="ob", bufs=2))
    rc_pool = ctx.enter_context(tc.tile_pool(name="rc", bufs=1))

    pt_tiles = {}   # (qc, mh) -> PT tile [P, 2, KT, CH]
    ot_tiles = {}   # qc -> OT tile [P, MT, CH]

    def proj_qk(c, is_q):
        xT = (xq_t if is_q else xk_t)[c]
        tgt = QT_sb if is_q else KT_sb
        b_sb = bq_sb if is_q else bk_sb
        w_sb = wq_sb if is_q else wk_sb
        for m in range(MT):
            ps = ps_proj.tile([P, CH], f32, tag="proj")
            for ft in range(FT):
                nc.tensor.matmul(
                    ps, w_sb[:, ft, m * P:(m + 1) * P], xT[:, ft, :],
                    start=(ft == 0), stop=(ft == FT - 1),
                )
            nc.vector.tensor_scalar_add(tgt[:, m, c, :], ps, b_sb[:, m:m + 1])

    def proj_v(c):
        xT = xv_t[c]
        for t4 in range(CH // P):
            ps = ps_proj.tile([P, F], f32, tag="proj")
            for ft in range(FT):
                nc.tensor.matmul(
                    ps, xT[:, ft, t4 * P:(t4 + 1) * P], wv_sb[:, ft, :],
                    start=(ft == 0), stop=(ft == FT - 1),
                )
            kt = c * (CH // P) + t4
            nc.vector.tensor_add(
                V_sb[:, kt, :, 0:DH],
                ps.rearrange("p (h d) -> p h d", h=HG),
                bv_sb.rearrange("p (h d) -> p h d", h=HG),
            )

    def scores(qc, mh, kts):
        pt = pt_tiles[(qc, mh)]
        for kt in kts:
            t0 = (kt % NCH) * P
            ps = ps_s.tile([P, 2, CH], f32, tag="s")
            nc.tensor.matmul(
                ps[:, 0, :], KT_sb[0:DH, mh, kt // NCH, t0:t0 + P],
                QT_sb[0:DH, mh, qc, :], start=True, stop=True,
            )
            nc.tensor.matmul(
                ps[:, 1, :], KT_sb[DH:P, mh, kt // NCH, t0:t0 + P],
                QT_sb[DH:P, mh, qc, :], start=True, stop=True,
            )
            pt_out = pt[:, :, kt, :]
            if kt in SCH_KTS:
                itmp = it_pool.tile([P, 2, CH], i32, tag="it")
                nc.vector.tensor_scalar(
                    itmp, ps, SCH_A, SCH_B,
                    op0=mybir.AluOpType.mult, op1=mybir.AluOpType.add,
                )
                nc.gpsimd.tensor_copy(pt_out, itmp.bitcast(f32))
            else:
                nc.scalar.activation(out=pt_out, in_=ps, func=EXP, scale=SCALE)

    po_tiles = {}  # (qc, h) -> open psum accumulator for a split attn_v

    def attn_v_part(qc, h, k0, k1):
        pt = pt_tiles[(qc, h // 2)]
        if k0 == 0:
            po_tiles[(qc, h)] = ps_o.tile([P, CH], f32, tag="o",
                                          name=f"po{qc}_{h}")
        po = po_tiles[(qc, h)]
        for kt in range(k0, k1):
            nc.tensor.matmul(
                po, V_sb[:, kt, h, :], pt[:, h % 2, kt, :],
                start=(kt == 0), stop=(kt == KT - 1),
                skip_group_check=True,
            )
        if k1 < KT:
            return
        mh, p0 = divmod(h, 2)
        p0 *= DH
        rs = rc_pool.tile([DH, CH], f32, tag="rs")
        rc = rc_pool.tile([DH, CH], f32, tag="rc")
        nc.vector.tensor_copy(rs, po[DH:P, :])
        nc.vector.reciprocal_approx_fast(rc, rs)
        nc.vector.tensor_mul(ot_tiles[qc][p0:p0 + DH, mh, :], po[0:DH, :], rc)

    def attn_v(qc, h):
        attn_v_part(qc, h, 0, KT)

    def out_proj(qc, t4):
        ot = ot_tiles[qc]
        ob = ob_pool.tile([P, D], bf16, tag="ob")
        for n2 in range(D // CH):
            ps = ps_proj.tile([P, CH], f32, tag="proj")
            for m in range(MT):
                nc.tensor.matmul(
                    ps, ot[:, m, t4 * P:(t4 + 1) * P],
                    wo_sb[:, m, n2 * CH:(n2 + 1) * CH],
                    start=(m == 0), stop=(m == MT - 1),
                )
            nc.vector.tensor_copy(ob[:, n2 * CH:(n2 + 1) * CH], ps)
        tt = qc * NCH + t4
        nc.sync.dma_start(out=out[tt * P:(tt + 1) * P, :], in_=ob)

    def new_pt(qc, mh):
        pt_tiles[(qc, mh)] = pt_pool.tile([P, 2, KT, CH], bf16, tag="pt",
                                          name=f"pt{qc}_{mh}")

    def new_ot(qc):
        ot_tiles[qc] = ot_pool.tile([P, MT, CH], bf16, tag="ot",
                                    name=f"ot{qc}")

    # ---- software-pipelined emission order ----
    # Pipeline unit = (qc, mh): 16 score-tile pairs (PE ~3.4us) feeding 16
    # exp ACTIVATEs (ScalarE ~16.5us). Filler work (projections, attn_v of
    # the previous unit, out-proj) is interleaved between score groups so
    # the in-order PE queue never blocks on the exp consumer.
    G = 4  # score k-tiles per emission group

    proj_k = lambda c: (lambda: proj_qk(c, False))
    proj_q = lambda c: (lambda: proj_qk(c, True))
    projv = lambda c: (lambda: proj_v(c))
    sc = lambda qc, mh, g: (
        lambda: scores(qc, mh, range(g * G, (g + 1) * G)))
    av = lambda qc, h: (lambda: attn_v(qc, h))
    avp = lambda qc, h, k0, k1: (lambda: attn_v_part(qc, h, k0, k1))
    op = lambda qc, t4: (lambda: out_proj(qc, t4))
    npt = lambda qc, mh: (lambda: new_pt(qc, mh))
    not_ = lambda qc: (lambda: new_ot(qc))

    schedule = [
        proj_k(0), proj_q(0), npt(0, 0), not_(0),
        # unit (0,0): K/V projections as filler
        sc(0, 0, 0), proj_k(1), sc(0, 0, 1), proj_k(2),
        sc(0, 0, 2), proj_k(3), sc(0, 0, 3), projv(0), npt(0, 1),
        # unit (0,1)
        sc(0, 1, 0), projv(1), sc(0, 1, 1), projv(2),
        sc(0, 1, 2), projv(3), sc(0, 1, 3), proj_q(1),
        npt(1, 0), not_(1),
        # unit (1,0): attn_v(0, h) gated on exp(0, mh) completion
        sc(1, 0, 0), av(0, 0), sc(1, 0, 1), av(0, 1),
        sc(1, 0, 2), proj_q(2), sc(1, 0, 3), npt(1, 1),
        # unit (1,1)
        sc(1, 1, 0), av(0, 2), sc(1, 1, 1), av(0, 3),
        sc(1, 1, 2), op(0, 0), op(0, 1), sc(1, 1, 3), op(0, 2), op(0, 3),
        npt(2, 0), not_(2),
        # unit (2,0)
        sc(2, 0, 0), av(1, 0), sc(2, 0, 1), av(1, 1),
        sc(2, 0, 2), proj_q(3), sc(2, 0, 3), npt(2, 1),
        # unit (2,1)
        sc(2, 1, 0), av(1, 2), sc(2, 1, 1), av(1, 3),
        sc(2, 1, 2), op(1, 0), op(1, 1), sc(2, 1, 3), op(1, 2), op(1, 3),
        npt(3, 0), not_(3),
        # unit (3,0)
        sc(3, 0, 0), av(2, 0), sc(3, 0, 1), av(2, 1),
        sc(3, 0, 2), av(2, 2), sc(3, 0, 3), npt(3, 1),
        # unit (3,1): attn_v(3, 0/1) overlaps exp(3, mh1); attn_v(3, 2/3)
        # accumulates split so only the last 4 k-tiles trail the final exp
        sc(3, 1, 0), av(2, 3), sc(3, 1, 1), av(3, 0),
        sc(3, 1, 2), av(3, 1), sc(3, 1, 3),
        op(2, 0), op(2, 1), op(2, 2), op(2, 3),
        avp(3, 2, 0, 12), avp(3, 3, 0, 12),
        # tail
        avp(3, 2, 12, KT), avp(3, 3, 12, KT),
        op(3, 0), op(3, 1), op(3, 2), op(3, 3),
    ]
    for unit in schedule:
        unit()


def _build():
    nc = bacc.Bacc("TRN2", target_bir_lowering=False, debug=False)
    # x inputs chunk-major [c, p, ft, t]; weights partition-major — every
    # DMA reads fully contiguous per-partition lines (host prepares these)
    xqT = nc.dram_tensor("xqT", [NCH, P, FT, CH], bf16,
                         kind="ExternalInput").ap()
    xkT = nc.dram_tensor("xkT", [NCH, P, FT, CH], bf16,
                         kind="ExternalInput").ap()
    xvT = nc.dram_tensor("xvT", [NCH, P, FT, CH], bf16,
                         kind="ExternalInput").ap()
    wqT = nc.dram_tensor("wqT", [P, FT, F], bf16, kind="ExternalInput").ap()
    wkT = nc.dram_tensor("wkT", [P, FT, F], bf16, kind="ExternalInput").ap()
    wvT = nc.dram_tensor("wvT", [P, FT, F], bf16, kind="ExternalInput").ap()
    woT = nc.dram_tensor("woT", [P, MT, D], bf16, kind="ExternalInput").ap()
    bq2 = nc.dram_tensor("bq2", [P, MT], f32, kind="ExternalInput").ap()
    bk2 = nc.dram_tensor("bk2", [P, MT], f32, kind="ExternalInput").ap()
    bv1 = nc.dram_tensor("bv1", [F], f32, kind="ExternalInput").ap()
    out = nc.dram_tensor("out", [S, D], bf16, kind="ExternalOutput").ap()
    from contextlib import ExitStack

    with tile.TileContext(nc) as tc, ExitStack() as ctx:
        _emit(ctx, nc, tc,
              (xqT, xkT, xvT, wqT, wkT, wvT, woT, bq2, bk2, bv1, out))
    nc.compile()
    nc.m = get_hw_module(nc.m)
    return nc


_cached_nc = None


def _get_nc():
    global _cached_nc
    if _cached_nc is None:
        _cached_nc = _build()
    return _cached_nc


def make_in_maps(query, key, value, Wq, bq, Wk, bk, Wv, bv, Wo, bo):
    query, key, value, Wq, bq, Wk, bk, Wv, bv, Wo = (
        np.asarray(a, np.float32)
        for a in (query, key, value, Wq, bq, Wk, bk, Wv, bv, Wo)
    )
    bff = ml_dtypes.bfloat16

    def x4(a, b):
        # [S, D] -> chunk-major [NCH, P, FT, CH] of a[b].T
        aT = a[b].T.reshape(FT, P, NCH, CH)
        return np.ascontiguousarray(aT.transpose(2, 1, 0, 3)).astype(bff)

    def w3(W, fs):
        # W[fs] is [F, D]; -> [P, FT, F] of W[fs].T
        wT = W[fs].T.reshape(FT, P, F)
        return np.ascontiguousarray(wT.transpose(1, 0, 2)).astype(bff)

    xTs = [
        tuple(x4(a, b) for a in (query, key, value)) for b in range(B)
    ]
    in_maps = []
    for c in range(N_CORES):
        b, g = divmod(c, 4)
        fs = slice(g * F, (g + 1) * F)
        qT, kT, vT = xTs[b]
        woT = Wo[:, fs].T.reshape(MT, P, D)
        in_maps.append({
            "xqT": qT,
            "xkT": kT,
            "xvT": vT,
            "wqT": w3(Wq, fs),
            "wkT": w3(Wk, fs),
            "wvT": w3(Wv, fs),
            "woT": np.ascontiguousarray(woT.transpose(1, 0, 2)).astype(bff),
            "bq2": np.ascontiguousarray(bq[fs].reshape(MT, P).T),
            "bk2": np.ascontiguousarray(bk[fs].reshape(MT, P).T),
            "bv1": np.ascontiguousarray(bv[fs]),
        })
    return in_maps


def combine_outputs(core_outs, bo):
    bo = np.asarray(bo, np.float32)
    out = np.empty((B, S, D), np.float32)
    for b in range(B):
        acc = core_outs[4 * b].astype(np.float32)
        for g in range(1, 4):
            acc = acc + core_outs[4 * b + g].astype(np.float32)
        out[b] = acc + bo
    return out


def kernel(query, key, value, Wq, bq, Wk, bk, Wv, bv, Wo, bo, **run_kwargs):
    nc = _get_nc()
    in_maps = make_in_maps(query, key, value, Wq, bq, Wk, bk, Wv, bv, Wo, bo)
    res = run_bass_kernel_spmd(
        nc, in_maps, core_ids=list(range(N_CORES)), **run_kwargs
    )
    out = combine_outputs([r["out"] for r in res.results], bo)
    if run_kwargs:
        kernel.last_results = res
    return out
